# revision 43
# baseline (speedup 1.0000x reference)
"""AllSet hypergraph NN (nn_AllSet_81020263071820) — Trainium2 Bass kernel, v2.

Self-contained: hardcodes shapes for N=100000 nodes, M=800000 incidences,
EH=50000 hyperedges, D=128, H=4 heads. Runs SPMD on 8 NeuronCores.

v2 strategy (vs the AllGather-table baseline):
- Replicate X (bf16) to every core; each core builds the FULL per-node
  message table locally (PE/DVE/ACT are idle anyway) — kills the 4-quarter
  table AllGather (~390us of serialized collective).
- Phase B (V2E) windows produce X1^T tiles; the two window-chunks are
  AllGathered as packed [128, rows] bf16 (1/4 the bytes of the padded
  table) and every core projects the full edge table locally (B2).
- Destination windows per core, dma_gather of 512B table rows into
  slot-buckets, one-hot fp8 matmul scatter-add into PSUM (as baseline),
  but gather calls are 2048-4096 idx (4x fewer GPSIMD SWDGE calls) and
  windows are processed in PAIRS to halve DVE/ACT instruction overheads.
"""
import sys

for _p in ("/opt/trn_rl_repo", "/root/.axon_site", "/root/.axon_site/_ro/pypackages"):
    if _p not in sys.path:
        sys.path.insert(0, _p)

import heapq
import numpy as np
import ml_dtypes

bf16 = ml_dtypes.bfloat16

N = 100000
M1 = 800001
EH1 = 50001
D = 128
H = 4
C = 32
NEG = 0.2
EPS = 1e-5
NCORES = 8
NPAD = 100352        # 784 node tiles; E2V destination rows
EPAD = 50176         # 392 edge windows; V2E destination rows
NW1 = 49             # V2E windows per core
NW2 = 98             # E2V windows per core
SH1 = EPAD // NCORES         # 6272 edge rows per core
SH2 = NPAD // NCORES         # 12544 node rows per core
QSIZE = NPAD // 4            # 25088 rows per table quarter (global)
# per-(core, quarter) block sizes: 128-aligned, alternating so every
# quarter sums to 25088 and every core's blocks sum to 12544
QC_KJ = np.array([[3200 if (k + j) % 2 == 0 else 3072 for j in range(4)]
                  for k in range(NCORES)])
QOFF_KJ = np.concatenate([np.zeros((NCORES, 1), np.int64),
                          np.cumsum(QC_KJ, axis=1)], axis=1)
QROW_KJ = np.concatenate([np.zeros((1, 4), np.int64),
                          np.cumsum(QC_KJ, axis=0)], axis=0)
BOUND = 3136                 # phase-C bucket boundary (local edge row)
CH_ROWS = [3200, 3072]       # x1 chunk cols per core (windows 0..24 / 25..48)
GW1 = 4
GW2 = 8
LIM = 512


# ---------------------------------------------------------------------------
# Host planning (graph-structure only; ported from the validated baseline)
# ---------------------------------------------------------------------------

def _plan_permutations(vtx, edg):
    """Returns pos3 (node->E2V row), pos2 (edge->V2E row),
    (kq, bq, iq) node table coords."""
    n_win2 = NPAD // 128
    deg_v0 = np.bincount(vtx, minlength=N)
    order_v0 = np.argsort(-deg_v0, kind="stable")
    heap3 = [(0, w) for w in range(n_win2)]
    heapq.heapify(heap3)
    w3count = np.zeros(n_win2, np.int64)
    w3tot = np.zeros(n_win2, np.int64)
    pos3 = np.zeros(N, np.int64)
    for v in order_v0:
        while True:
            t, w = heapq.heappop(heap3)
            if w3count[w] < 128:
                break
        pos3[v] = w * 128 + w3count[w]
        w3count[w] += 1
        w3tot[w] += deg_v0[v]
        heapq.heappush(heap3, (int(w3tot[w]), w))

    deg_e = np.bincount(edg, minlength=EH1)
    n_win1 = EPAD // 128
    n_core1 = NW1
    bound = BOUND
    order_e = np.argsort(-deg_e, kind="stable")
    order_ie = np.argsort(edg, kind="stable")
    e_sorted_v = vtx[order_ie]
    e_starts = np.searchsorted(edg[order_ie], np.arange(EH1 + 1))
    cell2 = np.zeros((n_win2, 2), np.int64)
    halfcap = [NCORES * bound, EPAD - NCORES * bound]
    halfcnt = [0, 0]
    half_of_e = np.zeros(EH1, np.int64)
    euw = [None] * EH1
    for e in order_e:
        vws = pos3[e_sorted_v[e_starts[e]:e_starts[e + 1]]] // 128
        uw, cnts = np.unique(vws, return_counts=True)
        euw[e] = (uw, cnts)
        if len(uw):
            o0 = np.maximum(cell2[uw, 0] + cnts - LIM, 0).sum()
            o1 = np.maximum(cell2[uw, 1] + cnts - LIM, 0).sum()
            m0 = (cell2[uw, 0] + cnts).max()
            m1 = (cell2[uw, 1] + cnts).max()
        else:
            o0 = o1 = m0 = m1 = 0
        if halfcnt[0] >= halfcap[0]:
            h = 1
        elif halfcnt[1] >= halfcap[1]:
            h = 0
        else:
            h = 0 if (o0, m0) <= (o1, m1) else 1
        half_of_e[e] = h
        halfcnt[h] += 1
        if len(uw):
            cell2[uw, h] += cnts
    w3_of_inc = pos3[vtx] // 128
    for _ in range(30):
        over = np.argwhere(cell2 > LIM)
        moved = 0
        for w3o, ho in over:
            if cell2[w3o, ho] <= LIM:
                continue
            cand = np.unique(edg[w3_of_inc == w3o])
            contr = []
            for e in cand:
                if half_of_e[e] != ho:
                    continue
                uw, cnts = euw[e]
                c = cnts[uw == w3o]
                contr.append((int(c[0]) if len(c) else 0, int(e)))
            contr.sort(reverse=True)
            for c, e in contr:
                if cell2[w3o, ho] <= LIM:
                    break
                hn = 1 - ho
                if halfcnt[hn] >= halfcap[hn]:
                    continue
                uw, cnts = euw[e]
                if (cell2[uw, hn] + cnts > LIM).any():
                    continue
                cell2[uw, ho] -= cnts
                cell2[uw, hn] += cnts
                halfcnt[ho] -= 1
                halfcnt[hn] += 1
                half_of_e[e] = hn
                moved += 1
        if len(over) == 0 or moved == 0:
            break
    # per-half LPT into windows (flex windows straddle the half boundary)
    heaps = {0: [], 1: []}
    bcap = {}
    for w in range(n_win1):
        lw = w % n_core1
        r0 = lw * 128
        c0 = min(max(bound - r0, 0), 128)
        if c0 > 0:
            bcap[(w, 0)] = c0
            heaps[0].append((0, w))
        if c0 < 128:
            bcap[(w, 1)] = 128 - c0
            heaps[1].append((0, w))
    for h in heaps:
        heapq.heapify(heaps[h])
    wtot = np.zeros(n_win1, np.int64)
    bctr = {k: 0 for k in bcap}
    pos2 = np.zeros(EH1, np.int64)
    for e in order_e:
        h = int(half_of_e[e])
        while True:
            t, w = heapq.heappop(heaps[h])
            if bctr[(w, h)] < bcap[(w, h)]:
                break
        base = 0 if h == 0 else bcap.get((w, 0), 0)
        pos2[e] = w * 128 + base + bctr[(w, h)]
        bctr[(w, h)] += 1
        wtot[w] += deg_e[e]
        heapq.heappush(heaps[h], (int(wtot[w]), w))
    # repair V2E window totals > 4*LIM (cannot split into 4 buckets)
    e_by_win = {}
    for e in range(EH1):
        e_by_win.setdefault(int(pos2[e]) // 128, []).append(e)
    for _ in range(20):
        over_ws = np.where(wtot > 4 * LIM)[0]
        if len(over_ws) == 0:
            break
        swapped = 0
        under_ws = np.argsort(wtot)[:64]
        for wo in over_ws:
            need = int(wtot[wo] - 4 * LIM)
            eo = sorted(e_by_win[int(wo)], key=lambda e: -deg_e[e])
            done = False
            for e1 in eo:
                h1_ = int(half_of_e[e1])
                for wu in under_ws:
                    if wtot[wu] > 4 * LIM - need:
                        continue
                    for e0 in e_by_win[int(wu)]:
                        if int(half_of_e[e0]) != h1_:
                            continue
                        d = int(deg_e[e1] - deg_e[e0])
                        if d >= need and wtot[wu] + d <= 4 * LIM:
                            pos2[e1], pos2[e0] = pos2[e0], pos2[e1]
                            e_by_win[int(wo)].remove(e1)
                            e_by_win[int(wu)].remove(e0)
                            e_by_win[int(wo)].append(e0)
                            e_by_win[int(wu)].append(e1)
                            wtot[wo] -= d
                            wtot[wu] += d
                            swapped += 1
                            done = True
                            break
                    if done:
                        break
                if done:
                    break
        if swapped == 0:
            break
    # greedy node -> quarter (cells (V2E window, quarter) <= LIM)
    dst1 = pos2[edg]
    w1_of_inc = dst1 // 128
    order_inc = np.argsort(vtx, kind="stable")
    v_sorted = vtx[order_inc]
    w_sorted = w1_of_inc[order_inc]
    starts = np.searchsorted(v_sorted, np.arange(N + 1))
    deg_v = starts[1:] - starts[:-1]
    cell = np.zeros((n_win1, 4), np.int32)
    cap_cell = np.zeros((NCORES, 4), np.int32)
    bucket_of_v = np.zeros(N, np.int64)
    order_v = np.argsort(-deg_v, kind="stable")
    for v in order_v:
        ws = w_sorted[starts[v]:starts[v + 1]]
        uw, cnts = np.unique(ws, return_counts=True)
        if len(uw):
            scores = (cell[uw] + cnts[:, None]).max(axis=0)
        else:
            scores = np.zeros(4, np.int64)
        for b in np.argsort(scores, kind="stable"):
            if (cap_cell[:, b] < QC_KJ[:, b]).any():
                break
        bucket_of_v[v] = b
        if len(uw):
            cell[uw, b] += cnts.astype(np.int32)
        k = int(np.argmin(np.where(cap_cell[:, b] < QC_KJ[:, b],
                                   cap_cell[:, b], 10 ** 9)))
        cap_cell[k, b] += 1
    for _ in range(30):
        overc = np.argwhere(cell > LIM)
        movedc = 0
        for w1o, bo in overc:
            if cell[w1o, bo] <= LIM:
                continue
            inc_sel = np.where(w1_of_inc == w1o)[0]
            cand = np.unique(vtx[inc_sel])
            contr = []
            for v in cand:
                if bucket_of_v[v] != bo:
                    continue
                ws = w_sorted[starts[v]:starts[v + 1]]
                contr.append((int((ws == w1o).sum()), int(v)))
            contr.sort(reverse=True)
            for c, v in contr:
                if cell[w1o, bo] <= LIM:
                    break
                ws = w_sorted[starts[v]:starts[v + 1]]
                uw, cnts = np.unique(ws, return_counts=True)
                for bn in range(4):
                    if bn == bo or not (cap_cell[:, bn] < QC_KJ[:, bn]).any():
                        continue
                    if (cell[uw, bn] + cnts > LIM).any():
                        continue
                    cell[uw, bo] -= cnts.astype(np.int32)
                    cell[uw, bn] += cnts.astype(np.int32)
                    ko = int(np.argmax(cap_cell[:, bo]))
                    cap_cell[ko, bo] -= 1
                    kn = int(np.argmin(np.where(
                        cap_cell[:, bn] < QC_KJ[:, bn],
                        cap_cell[:, bn], 10 ** 9)))
                    cap_cell[kn, bn] += 1
                    bucket_of_v[v] = bn
                    movedc += 1
                    break
        if len(overc) == 0 or movedc == 0:
            break
    kq = np.zeros(N, np.int64)
    iq = np.zeros(N, np.int64)
    cap_cell2 = np.zeros((NCORES, 4), np.int64)
    for v in order_v:
        b = bucket_of_v[v]
        k = int(np.argmin(np.where(cap_cell2[:, b] < QC_KJ[:, b],
                                   cap_cell2[:, b], 10 ** 9)))
        kq[v] = k
        iq[v] = cap_cell2[k, b]
        cap_cell2[k, b] += 1
    return pos3, pos2, kq, bucket_of_v, iq, int(cell.max()), int(cell2.max())


def _make_plan(dst, bkt, pos, n_dst_pad, qsizes, gw):
    dst = np.asarray(dst, np.int64)
    bkt = np.asarray(bkt, np.int64)
    pos = np.asarray(pos, np.int64)
    nb = len(qsizes)
    assert all(q < 32768 for q in qsizes)
    per_core = n_dst_pad // NCORES
    n_win = per_core // 128
    core_of = dst // per_core
    win_of = (dst % per_core) // 128
    counts = np.zeros((NCORES, n_win, nb), np.int64)
    np.add.at(counts, (core_of, win_of, bkt), 1)
    cap = int(np.ceil(max(counts.max(), 1) / 128) * 128)
    cpw = cap // 128
    groups = []
    w = 0
    while w < n_win:
        groups.append(min(gw, n_win - w))
        w += gw
    gpre = np.concatenate([[0], np.cumsum(groups)])
    total_slots = n_win * nb * cap
    total_chunks = total_slots // 128
    g_of_win = np.zeros(n_win, np.int64)
    wig_of_win = np.zeros(n_win, np.int64)
    for g, gs in enumerate(groups):
        for wi in range(gs):
            g_of_win[gpre[g] + wi] = g
            wig_of_win[gpre[g] + wi] = wi
    key = (core_of * n_win + win_of) * nb + bkt
    order = np.argsort(key, kind="stable")
    key_s = key[order]
    cell_sizes = np.bincount(key_s, minlength=NCORES * n_win * nb)
    cell_starts = np.concatenate([[0], np.cumsum(cell_sizes)])
    p = np.arange(len(key_s)) - cell_starts[key_s]
    co = core_of[order]
    wo = win_of[order]
    bo = bkt[order]
    gg = g_of_win[wo]
    wig = wig_of_win[wo]
    gs_arr = np.asarray(groups)[gg]
    slot = (gpre[gg] * nb + bo * gs_arr) * cap + wig * cap + p
    idx16 = np.zeros((NCORES, total_slots), np.int16)
    ids = np.full((NCORES, total_chunks, 128), -1.0, np.float32)
    idx16[co, slot] = pos[order].astype(np.int16)
    ids[co, slot // 128, slot % 128] = (dst[order] % 128).astype(np.float32)
    wrapped = idx16.reshape(NCORES, total_slots // 16, 16).transpose(0, 2, 1)
    idx_up = np.tile(wrapped, (1, 8, 1)).astype(np.int16)
    oh = (ids[:, :, :, None] ==
          np.arange(128, dtype=np.float32)[None, None, None, :])
    oh_up = np.ascontiguousarray(
        oh.transpose(0, 2, 1, 3).reshape(NCORES, 128, total_chunks * 128)
    ).astype(ml_dtypes.float8_e4m3)
    return dict(cap=cap, cpw=cpw, n_win=n_win, groups=[int(x) for x in groups],
                gpre=[int(x) for x in gpre], nb=nb,
                idx_up=idx_up, oh_up=oh_up, qsizes=list(qsizes),
                total_slots=total_slots, total_chunks=total_chunks)


def _proj_weights(Kw, Kb, Vw, Vb, att):
    att_f = np.asarray(att, np.float32).reshape(H, C)
    Kw_a = np.zeros((D, H), np.float32)
    Kb_a = np.zeros((H,), np.float32)
    for h in range(H):
        Kw_a[:, h] = np.asarray(Kw, np.float32)[:, h * C:(h + 1) * C] @ att_f[h]
        Kb_a[h] = np.asarray(Kb, np.float32)[h * C:(h + 1) * C] @ att_f[h]
    pw = np.concatenate([np.asarray(Vw, np.float32), Kw_a], axis=1)
    pb = np.concatenate([np.asarray(Vb, np.float32), Kb_a])
    return pw, pb


# ---------------------------------------------------------------------------
# Device graph
# ---------------------------------------------------------------------------

def _build_nc(plan1, plan2):
    import os
    import concourse.bass as bass
    import concourse.bacc as bacc
    import concourse.mybir as mybir
    import concourse.tile as tile

    dt = mybir.dt
    Alu = mybir.AluOpType
    Act = mybir.ActivationFunctionType

    from concourse.hw_specs import get_activation_tables

    nc = bacc.Bacc("TRN2", target_bir_lowering=False, debug=False,
                   num_devices=NCORES, num_swdge_queues=4)
    _tabs = get_activation_tables(nc.m.arch)
    for _k, _v in _tabs.items():
        if _k != "natural_log_exp_and_others":
            _v.clear()

    def ein(name, shape, dty):
        return nc.dram_tensor(name, shape, dty, kind="ExternalInput")

    xt = ein("xt", [128, NPAD], dt.bfloat16)
    pw1 = ein("pw1", [128, 132], dt.bfloat16)
    pw2 = ein("pw2", [128, 132], dt.bfloat16)
    ff1_1 = ein("ff1_1", [128, 128], dt.bfloat16)
    ff2_1 = ein("ff2_1", [128, 128], dt.bfloat16)
    ff1_2 = ein("ff1_2", [128, 128], dt.bfloat16)
    ff2_2 = ein("ff2_2", [128, 128], dt.bfloat16)
    b1c_1 = ein("b1c_1", [128, 1], dt.float32)
    b2c_1 = ein("b2c_1", [128, 1], dt.float32)
    b1c_2 = ein("b1c_2", [128, 1], dt.float32)
    b2c_2 = ein("b2c_2", [128, 1], dt.float32)
    att1 = ein("att1", [128, 256], dt.bfloat16)
    att2 = ein("att2", [128, 256], dt.bfloat16)
    ident = ein("ident", [128, 128], dt.bfloat16)
    epsc = ein("epsc", [128, 1], dt.float32)
    idx1 = ein("idx1", [128, plan1["total_slots"] // 16], dt.int16)
    idx2 = ein("idx2", [128, plan2["total_slots"] // 16], dt.int16)
    oh1 = ein("oh1", [128, plan1["total_chunks"] * 128], dt.float8e4)
    oh2 = ein("oh2", [128, plan2["total_chunks"] * 128], dt.float8e4)
    out = nc.dram_tensor("out", [SH2, 128], dt.float32,
                         kind="ExternalOutput")

    tbl1q = [nc.dram_tensor(f"tbl1q{j}", [QSIZE, 256], dt.bfloat16)
             for j in range(4)]
    x1t_sh = [nc.dram_tensor(f"x1t_sh{c}", [128, CH_ROWS[c]], dt.bfloat16)
              for c in range(2)]
    x1t_f = [nc.dram_tensor(f"x1t_f{c}", [NCORES * 128, CH_ROWS[c]],
                            dt.bfloat16)
             for c in range(2)]
    tbl2b = [nc.dram_tensor(f"tbl2b{c}", [NCORES * BOUND, 256],
                            dt.bfloat16) for c in range(2)]

    cap1, cpw1 = plan1["cap"], plan1["cpw"]
    cap2, cpw2 = plan2["cap"], plan2["cpw"]
    groups1, gpre1 = plan1["groups"], plan1["gpre"]
    groups2, gpre2 = plan2["groups"], plan2["gpre"]

    with tile.TileContext(nc) as tc:
        with tc.tile_pool(name="const", bufs=1) as cp:
            def load_const(name, src_ap, shape, dty):
                t = cp.tile(shape, dty, tag=name)
                nc.sync.dma_start(t[:], src_ap)
                return t

            pw1_t = load_const("pw1", pw1[:], [128, 132], dt.bfloat16)
            pw2_t = load_const("pw2", pw2[:], [128, 132], dt.bfloat16)
            ff11_t = load_const("ff11", ff1_1[:], [128, 128], dt.bfloat16)
            ff21_t = load_const("ff21", ff2_1[:], [128, 128], dt.bfloat16)
            ff12_t = load_const("ff12", ff1_2[:], [128, 128], dt.bfloat16)
            ff22_t = load_const("ff22", ff2_2[:], [128, 128], dt.bfloat16)
            b11_t = load_const("b11", b1c_1[:], [128, 1], dt.float32)
            b21_t = load_const("b21", b2c_1[:], [128, 1], dt.float32)
            b12_t = load_const("b12", b1c_2[:], [128, 1], dt.float32)
            b22_t = load_const("b22", b2c_2[:], [128, 1], dt.float32)
            att1_t = load_const("att1", att1[:], [128, 256], dt.bfloat16)
            att2_t = load_const("att2", att2[:], [128, 256], dt.bfloat16)
            ident_t = load_const("ident", ident[:], [128, 128], dt.bfloat16)
            eps_t = load_const("epsc", epsc[:], [128, 1], dt.float32)
            idx1_t = cp.tile([128, plan1["total_slots"] // 16], dt.int16,
                             tag="idx1")
            nc.sync.dma_start(idx1_t[:], idx1[:])
            idx2_t = cp.tile([128, plan2["total_slots"] // 16], dt.int16,
                             tag="idx2")
            nc.sync.dma_start(idx2_t[:], idx2[:])

            # ---------------- shared helpers --------------------------------
            def build_rows(pool, psp, xsrc, dst_write, pw_t, n, rows=128):
                """Project n (<=3) row-tiles (last may have rows<128):
                psum=[x@Vw | alpha], w=exp(leaky(alpha)), write [xV*w | w].
                xsrc: SBUF AP holding the (n-1)*128+rows source columns."""
                ps = psp.tile([128, 396], dt.float32, tag="bps")
                for j in range(n):
                    r = 128 if j < n - 1 else rows
                    nc.tensor.matmul(ps[0:r, j * 132:(j + 1) * 132],
                                     xsrc[:, j * 128:j * 128 + r],
                                     pw_t[:], start=True, stop=True)
                tb = pool.tile([128, 768], dt.bfloat16, tag="btb")
                tb3 = tb.rearrange("p (t c) -> p t c", c=256)
                ps3 = ps.rearrange("p (t c) -> p t c", c=132)
                w4a = pool.tile([128, 12], dt.float32, tag="bw4a")
                w4av = w4a.rearrange("p (t c) -> p t c", c=4)[:, 0:n, :]
                w4 = pool.tile([128, 12], dt.float32, tag="bw4")
                w4v = w4.rearrange("p (t c) -> p t c", c=4)[:, 0:n, :]
                a_ap = ps3[:, 0:n, 128:132]
                nc.vector.tensor_scalar(w4av, a_ap, NEG, None, Alu.mult)
                nc.vector.tensor_tensor(w4v, w4av, a_ap, Alu.max)
                nc.scalar.activation(tb3[:, 0:n, 128:132], w4v, Act.Exp)
                nc.vector.tensor_tensor(
                    tb3[:, 0:n, 0:128].rearrange("p t (h c2) -> p t h c2",
                                                 c2=C),
                    ps3[:, 0:n, 0:128].rearrange("p t (h c2) -> p t h c2",
                                                 c2=C),
                    tb3[:, 0:n, 128:132].unsqueeze(-1).broadcast_to(
                        [128, n, H, C]),
                    Alu.mult)
                dst_write(tb)

            def pma_pair(pss, n, att_t, ff1_t, ff2_t, b1_t, b2_t, pool, psp,
                         final):
                """PMA epilogue for n (1 or 2) windows, each with its own
                [128,132] psum tile.  Returns [128, n*128] tile."""
                z2 = pool.tile([128, n * 128], dt.bfloat16, tag="z2")
                z23 = z2.rearrange("p (t c) -> p t c", c=128)
                den = pool.tile([128, 8], dt.float32, tag="den")
                rec = pool.tile([128, 8], dt.float32, tag="rec")
                zt = pool.tile([128, n * 128], dt.bfloat16, tag="zt")
                for t in range(n):
                    nc.vector.tensor_scalar(den[:, t * 4:t * 4 + 4],
                                            pss[t][:, 128:132],
                                            1e-16, None, Alu.add)
                    nc.vector.reciprocal(rec[:, t * 4:t * 4 + 4],
                                         den[:, t * 4:t * 4 + 4])
                    nc.vector.tensor_tensor(
                        zt[:, t * 128:(t + 1) * 128].rearrange(
                            "p (h c2) -> p h c2", c2=C),
                        pss[t][:, 0:128].rearrange(
                            "p (h c2) -> p h c2", c2=C),
                        rec[:, t * 4:t * 4 + 4].unsqueeze(-1).broadcast_to(
                            [128, H, C]),
                        Alu.mult)
                nc.vector.tensor_tensor(z2[:, 0:n * 128], zt[:, 0:n * 128],
                                        att_t[:, 0:n * 128], Alu.add)
                st = pool.tile([128, n * 6], dt.float32, tag="st")
                stg = st.rearrange("p (t c) -> p t c", c=6)
                for t in range(n):
                    nc.vector.bn_stats(stg[:, t, :], z23[:, t, :])
                mv = pool.tile([128, n * 2], dt.float32, tag="mv")
                for t in range(n):
                    nc.vector.bn_aggr(mv[:, t * 2:t * 2 + 2], stg[:, t, :])
                lv = pool.tile([128, 2], dt.float32, tag="lv")
                rstd = pool.tile([128, 2], dt.float32, tag="rstd")
                nmr = pool.tile([128, 2], dt.float32, tag="nmr")
                u = pool.tile([128, n * 128], dt.bfloat16, tag="u")
                u3 = u.rearrange("p (t c) -> p t c", c=128)
                for t in range(n):
                    nc.scalar.activation(lv[:, t:t + 1],
                                         mv[:, t * 2 + 1:t * 2 + 2], Act.Ln,
                                         bias=eps_t[:])
                    nc.scalar.activation(rstd[:, t:t + 1], lv[:, t:t + 1],
                                         Act.Exp, scale=-0.5)
                    nc.vector.tensor_scalar(u3[:, t, :], z23[:, t, :],
                                            mv[:, t * 2:t * 2 + 1],
                                            rstd[:, t:t + 1],
                                            Alu.subtract, Alu.mult)
                ptx = psp.tile([128, n * 128], dt.bfloat16, tag="trps")
                for t in range(n):
                    nc.tensor.transpose(ptx[:, t * 128:(t + 1) * 128],
                                        u3[:, t, :], ident_t[:])
                uT = pool.tile([128, n * 128], dt.bfloat16, tag="uT")
                nc.vector.tensor_copy(uT[:], ptx[:])
                pf1 = psp.tile([128, n * 128], dt.float32, tag="mmps")
                nc.tensor.matmul(pf1[:], ff1_t[:], uT[:], start=True,
                                 stop=True)
                f1 = pool.tile([128, n * 128], dt.bfloat16, tag="f1")
                nc.scalar.activation(f1[:], pf1[:], Act.Relu, bias=b1_t[:])
                pf2 = psp.tile([128, n * 128], dt.float32, tag="mmps")
                nc.tensor.matmul(pf2[:], ff2_t[:], f1[:], start=True,
                                 stop=True)
                f2T = pool.tile([128, n * 128], dt.bfloat16, tag="f2T")
                nc.scalar.activation(f2T[:], pf2[:], Act.Relu, bias=b2_t[:])
                pt2 = psp.tile([128, n * 128], dt.bfloat16, tag="trps")
                for t in range(n):
                    nc.tensor.transpose(pt2[:, t * 128:(t + 1) * 128],
                                        f2T[:, t * 128:(t + 1) * 128],
                                        ident_t[:])
                r = pool.tile([128, n * 128], dt.bfloat16, tag="r")
                nc.vector.tensor_tensor(r[:], pt2[:], u[:], Alu.add)
                r3 = r.rearrange("p (t c) -> p t c", c=128)
                st2 = pool.tile([128, n * 6], dt.float32, tag="st")
                st2g = st2.rearrange("p (t c) -> p t c", c=6)
                for t in range(n):
                    nc.vector.bn_stats(st2g[:, t, :], r3[:, t, :])
                mv2 = pool.tile([128, n * 2], dt.float32, tag="mv")
                lv2 = pool.tile([128, 2], dt.float32, tag="lv")
                rstd2 = pool.tile([128, 2], dt.float32, tag="rstd")
                nmr2 = pool.tile([128, 2], dt.float32, tag="nmr")
                for t in range(n):
                    nc.vector.bn_aggr(mv2[:, t * 2:t * 2 + 2], st2g[:, t, :])
                    nc.scalar.activation(lv2[:, t:t + 1],
                                         mv2[:, t * 2 + 1:t * 2 + 2],
                                         Act.Ln, bias=eps_t[:])
                    nc.scalar.activation(rstd2[:, t:t + 1], lv2[:, t:t + 1],
                                         Act.Exp, scale=-0.5)
                    nc.vector.tensor_scalar(nmr2[:, t:t + 1],
                                            mv2[:, t * 2:t * 2 + 1],
                                            rstd2[:, t:t + 1], -1.0,
                                            Alu.mult, Alu.mult)
                if final:
                    o = pool.tile([128, n * 128], dt.float32, tag="fin")
                    o3 = o.rearrange("p (t c) -> p t c", c=128)
                    for t in range(n):
                        nc.scalar.activation(o3[:, t, :], r3[:, t, :],
                                             Act.Identity,
                                             bias=nmr2[:, t:t + 1],
                                             scale=rstd2[:, t:t + 1])
                    return o
                x1 = pool.tile([128, n * 128], dt.bfloat16, tag="x1")
                x13 = x1.rearrange("p (t c) -> p t c", c=128)
                for t in range(n):
                    nc.scalar.activation(x13[:, t, :], r3[:, t, :],
                                         Act.Relu, bias=nmr2[:, t:t + 1],
                                         scale=rstd2[:, t:t + 1])
                return x1

            # ---------------- phase A: full local table1 --------------------
            with tc.tile_pool(name="pa", bufs=6) as pa, \
                 tc.tile_pool(name="pa_ps", bufs=3, space="PSUM") as paps:
                for k in range(NCORES):
                    for j in range(4):
                        ntile = int(QC_KJ[k, j]) // 128
                        tq0 = 0
                        while tq0 < ntile:
                            nt12 = min(12, ntile - tq0)
                            col0 = k * SH2 + int(QOFF_KJ[k, j]) + tq0 * 128
                            xbig = pa.tile([128, 1536], dt.bfloat16,
                                           tag="bxt")
                            nc.sync.dma_start(xbig[:, 0:nt12 * 128],
                                              xt[:, col0:col0 + nt12 * 128])
                            tq = tq0
                            while tq < tq0 + nt12:
                                n = min(3, tq0 + nt12 - tq)

                                def writet(tb, j=j, k=k, tq=tq, n=n):
                                    r0 = int(QROW_KJ[k, j]) + tq * 128
                                    dst = tbl1q[j][r0:r0 + n * 128, :]
                                    eng = (nc.scalar if (tq // 3) % 2
                                           else nc.sync)
                                    eng.dma_start(
                                        dst.rearrange("(t p) c -> p t c",
                                                      p=128),
                                        tb.rearrange("p (t c) -> p t c",
                                                     c=256)[:, 0:n, :])

                                off = (tq - tq0) * 128
                                build_rows(pa, paps,
                                           xbig[:, off:off + n * 128],
                                           writet, pw1_t, n)
                                tq += n
                            tq0 += nt12

            # ---------------- phase B: V2E ----------------------------------
            with tc.tile_pool(name="pbg", bufs=2) as pbg, \
                 tc.tile_pool(name="pbo", bufs=2) as pbo, \
                 tc.tile_pool(name="pb", bufs=4) as pb, \
                 tc.tile_pool(name="pb_ps", bufs=2, space="PSUM") as pbps, \
                 tc.tile_pool(name="pb_ag", bufs=2, space="PSUM") as pbag:
                qcount = 0
                for g, gs in enumerate(groups1):
                    nch_g = gs * 4 * cpw1
                    ch0 = gpre1[g] * 4 * cpw1
                    poh = pbo.tile([128, GW1 * 4 * cpw1 * 128], dt.float8e4,
                                   tag="poh")
                    nc.sync.dma_start(poh[:, 0:nch_g * 128],
                                      oh1[:, ch0 * 128:(ch0 + nch_g) * 128])
                    gbs = []
                    for b in range(4):
                        s0 = (gpre1[g] * 4 + b * gs) * cap1
                        nidx = gs * cap1
                        gb = pbg.tile([128, GW1 * cap1 * 2], dt.bfloat16,
                                      tag=f"gb{b}")
                        done = 0
                        while done < nidx:
                            nn_ = min(1024, nidx - done)
                            nc.gpsimd.dma_gather(
                                gb[:, done * 2:(done + nn_) * 2].rearrange(
                                    "p (k e) -> p k e", e=256),
                                tbl1q[b][:, :],
                                idx1_t[:, (s0 + done) // 16:
                                       (s0 + done + nn_) // 16],
                                nn_, nn_, 256, queue_num=qcount % 4)
                            qcount += 1
                            done += nn_
                        gbs.append(gb)
                    wig = 0
                    while wig < gs:
                        n = min(2, gs - wig)
                        ps_a = pbag.tile([128, 512], dt.float32,
                                         tag="agg0")
                        ps_b = pbag.tile([128, 512], dt.float32,
                                         tag="agg1")
                        pss = [ps_a, ps_b][0:n]
                        for b in range(4):
                            for t in range(n):
                                for cc in range(cpw1):
                                    lch = (b * gs + wig + t) * cpw1 + cc
                                    blk = (wig + t) * cpw1 + cc
                                    nc.tensor.matmul(
                                        pss[t][:, 0:132],
                                        poh[:, lch * 128:(lch + 1) * 128],
                                        gbs[b][:, blk * 256:blk * 256 + 132],
                                        start=(b == 0 and cc == 0),
                                        stop=(b == 3 and cc == cpw1 - 1))
                        x1 = pma_pair(pss, n, att1_t, ff11_t, ff21_t, b11_t,
                                      b21_t, pb, pbps, final=False)
                        ptx = pbps.tile([128, n * 128], dt.bfloat16,
                                        tag="trps")
                        x13 = x1.rearrange("p (t c) -> p t c", c=128)
                        for t in range(n):
                            nc.tensor.transpose(ptx[:, t * 128:(t + 1) * 128],
                                                x13[:, t, :], ident_t[:])
                        x1T = pb.tile([128, n * 128], dt.bfloat16, tag="x1T")
                        nc.vector.tensor_copy(x1T[:], ptx[:])
                        lw0 = gpre1[g] + wig
                        if lw0 < 25:
                            # windows 24,25 pair straddles the chunk bound
                            n0 = min(n, 25 - lw0)
                            nc.sync.dma_start(
                                x1t_sh[0][:, lw0 * 128:(lw0 + n0) * 128],
                                x1T[:, 0:n0 * 128])
                            if n0 < n:
                                nc.sync.dma_start(
                                    x1t_sh[1][:, 0:(n - n0) * 128],
                                    x1T[:, n0 * 128:n * 128])
                        else:
                            lcol = (lw0 - 25) * 128
                            nc.sync.dma_start(
                                x1t_sh[1][:, lcol:lcol + n * 128], x1T[:])
                        if lw0 + n == 25 or (lw0 < 25 <= lw0 + n):
                            nc.gpsimd.collective_compute(
                                "AllGather", Alu.bypass,
                                replica_groups=[list(range(NCORES))],
                                ins=[x1t_sh[0].ap().opt()],
                                outs=[x1t_f[0].ap().opt()])
                        wig += n
                nc.gpsimd.collective_compute(
                    "AllGather", Alu.bypass,
                    replica_groups=[list(range(NCORES))],
                    ins=[x1t_sh[1].ap().opt()],
                    outs=[x1t_f[1].ap().opt()])

            # ---------------- phase B2: full local table2 -------------------
            # tbl2b[h] rows k*3136 + i; source x1 local row l = h*3136 + i:
            # l < 3200 -> x1t_f[0][k,:,l], else x1t_f[1][k,:,l-3200]
            with tc.tile_pool(name="pb2", bufs=6) as pb2, \
                 tc.tile_pool(name="pb2_ps", bufs=3, space="PSUM") as pb2ps:
                for h in range(2):
                    for k in range(NCORES):
                        segs = []          # (row0, nrows-per-tile list, ...)
                        i = 0
                        while i < BOUND:
                            l = h * BOUND + i
                            if l < 3200:
                                lim_seg = min(3200 - l, BOUND - i)
                                src = 0
                                scol = l
                            else:
                                lim_seg = BOUND - i
                                src = 1
                                scol = l - 3200
                            take = min(384, lim_seg)
                            ntile = (take + 127) // 128
                            lastr = take - (ntile - 1) * 128

                            x2t = pb2.tile([128, 384], dt.bfloat16,
                                           tag="b2x")
                            nc.sync.dma_start(
                                x2t[:, 0:take],
                                x1t_f[src][k * 128:(k + 1) * 128,
                                           scol:scol + take])

                            def writet2(tb, h=h, k=k, i=i, ntile=ntile,
                                        lastr=lastr):
                                r0 = k * BOUND + i
                                nf = ntile if lastr == 128 else ntile - 1
                                if nf:
                                    dst = tbl2b[h][r0:r0 + nf * 128, :]
                                    nc.scalar.dma_start(
                                        dst.rearrange("(t p) c -> p t c",
                                                      p=128),
                                        tb.rearrange("p (t c) -> p t c",
                                                     c=256)[:, 0:nf, :])
                                if nf < ntile:
                                    nc.scalar.dma_start(
                                        tbl2b[h][r0 + nf * 128:
                                                 r0 + nf * 128 + lastr, :],
                                        tb[0:lastr,
                                           nf * 256:(nf + 1) * 256])

                            build_rows(pb2, pb2ps, x2t[:, 0:take],
                                       writet2, pw2_t, ntile, rows=lastr)
                            i += take

            # ---------------- phase C: E2V ----------------------------------
            with tc.tile_pool(name="pcg", bufs=2) as pcg, \
                 tc.tile_pool(name="pco", bufs=2) as pco, \
                 tc.tile_pool(name="pc", bufs=4) as pc, \
                 tc.tile_pool(name="pc_ps", bufs=2, space="PSUM") as pcps, \
                 tc.tile_pool(name="pc_ag", bufs=2, space="PSUM") as pcag:
                qcount = 0
                for g, gs in enumerate(groups2):
                    nch_g = gs * 2 * cpw2
                    ch0 = gpre2[g] * 2 * cpw2
                    poh = pco.tile([128, GW2 * 2 * cpw2 * 128], dt.float8e4,
                                   tag="poh2")
                    nc.sync.dma_start(poh[:, 0:nch_g * 128],
                                      oh2[:, ch0 * 128:(ch0 + nch_g) * 128])
                    gbs = []
                    for b in range(2):
                        s0 = (gpre2[g] * 2 + b * gs) * cap2
                        nidx = gs * cap2
                        gb = pcg.tile([128, GW2 * cap2 * 2], dt.bfloat16,
                                      tag=f"gc{b}")
                        done = 0
                        while done < nidx:
                            nn_ = min(1024, nidx - done)
                            nc.gpsimd.dma_gather(
                                gb[:, done * 2:(done + nn_) * 2].rearrange(
                                    "p (k e) -> p k e", e=256),
                                tbl2b[b][:, :],
                                idx2_t[:, (s0 + done) // 16:
                                       (s0 + done + nn_) // 16],
                                nn_, nn_, 256, queue_num=qcount % 4)
                            qcount += 1
                            done += nn_
                        gbs.append(gb)
                    wig = 0
                    while wig < gs:
                        n = min(2, gs - wig)
                        ps_a = pcag.tile([128, 512], dt.float32,
                                         tag="agg20")
                        ps_b = pcag.tile([128, 512], dt.float32,
                                         tag="agg21")
                        pss = [ps_a, ps_b][0:n]
                        for b in range(2):
                            for t in range(n):
                                for cc in range(cpw2):
                                    lch = (b * gs + wig + t) * cpw2 + cc
                                    blk = (wig + t) * cpw2 + cc
                                    nc.tensor.matmul(
                                        pss[t][:, 0:132],
                                        poh[:, lch * 128:(lch + 1) * 128],
                                        gbs[b][:, blk * 256:blk * 256 + 132],
                                        start=(b == 0 and cc == 0),
                                        stop=(b == 1 and cc == cpw2 - 1))
                        o = pma_pair(pss, n, att2_t, ff12_t, ff22_t, b12_t,
                                     b22_t, pc, pcps, final=True)
                        w0 = gpre2[g] + wig
                        for t in range(n):
                            nc.sync.dma_start(
                                out[(w0 + t) * 128:(w0 + t + 1) * 128, :],
                                o[:, t * 128:(t + 1) * 128])
                        wig += n

    nc.finalize()
    return nc


# ---------------------------------------------------------------------------
# Entry point
# ---------------------------------------------------------------------------

_cache = {}
last_result = None


def kernel(**inputs):
    import os
    from concourse.bass_utils import run_bass_kernel_spmd

    X = np.asarray(inputs["X"], np.float32)
    vertex = np.asarray(inputs["vertex"], np.int64)
    edges = np.asarray(inputs["edges"], np.int64)
    vtx = np.concatenate([vertex, [N - 1]])
    edg = np.concatenate([edges, [EH1 - 1]])

    def P(prefix):
        return {k: np.asarray(inputs[f"{prefix}_{k}"], np.float32)
                for k in ("Kw", "Kb", "Vw", "Vb", "att", "w1", "b1", "w2",
                          "b2", "ln0s", "ln0b", "ln1s", "ln1b")}

    p1, p2 = P("v2e"), P("e2v")

    import hashlib
    hsh = hashlib.md5(vtx.tobytes() + edg.tobytes()).hexdigest()[:12]
    pcache = f"/tmp/allset_plan_{hsh}.npz"
    try:
        zc = np.load(pcache)
        pos3, pos2, kq, bq, iq = (zc["pos3"], zc["pos2"], zc["kq"],
                                  zc["bq"], zc["iq"])
        cm1 = cm2 = -1
    except Exception:
        pos3, pos2, kq, bq, iq, cm1, cm2 = _plan_permutations(vtx, edg)
        try:
            np.savez(pcache, pos3=pos3, pos2=pos2, kq=kq, bq=bq, iq=iq)
        except Exception:
            pass
    print(f"planner cellmax: phaseB={cm1} phaseC={cm2}", file=sys.stderr)

    pos1 = QROW_KJ[kq, bq] + iq                  # row within quarter tensor
    plan1 = _make_plan(pos2[edg], bq[vtx], pos1[vtx],
                       EPAD, [QSIZE] * 4, GW1)
    k_of = pos2[edg] // SH1
    l_of = pos2[edg] % SH1
    bkt2 = (l_of >= BOUND).astype(np.int64)
    pos_t2 = k_of * BOUND + (l_of % BOUND)
    plan2 = _make_plan(pos3[vtx], bkt2, pos_t2, NPAD,
                       [NCORES * BOUND] * 2, GW2)
    print(f"plan caps: V2E={plan1['cap']} E2V={plan2['cap']} "
          f"slots={plan1['total_slots']}/{plan2['total_slots']}",
          file=sys.stderr)

    pw_1, pb_1 = _proj_weights(p1["Kw"], p1["Kb"], p1["Vw"], p1["Vb"],
                               p1["att"])
    pw_2, pb_2 = _proj_weights(p2["Kw"], p2["Kb"], p2["Vw"], p2["Vb"],
                               p2["att"])
    assert np.all(pb_1 == 0) and np.all(pb_2 == 0), \
        "nonzero projection biases not supported by this kernel build"
    for p in (p1, p2):
        assert np.all(p["ln0s"] == 1) and np.all(p["ln0b"] == 0)
        assert np.all(p["ln1s"] == 1) and np.all(p["ln1b"] == 0)
        assert np.all(p["b1"] == 0) and np.all(p["b2"] == 0)

    ff1_1 = (np.diag(p1["ln0s"]) @ p1["w1"]).astype(bf16)
    ff1_2 = (np.diag(p2["ln0s"]) @ p2["w1"]).astype(bf16)
    b1_1 = (p1["ln0b"] @ p1["w1"] + p1["b1"]).astype(np.float32)
    b1_2 = (p2["ln0b"] @ p2["w1"] + p2["b1"]).astype(np.float32)

    # XT column for node v: core kq, quarter bq at QOFF_KJ, index iq
    xcol = kq * SH2 + QOFF_KJ[kq, bq] + iq
    XT = np.zeros((128, NPAD), np.float32)
    XT[:, xcol] = X.T
    XTb = XT.astype(bf16)
    ident = np.eye(128, dtype=np.float32)

    in_maps = []
    for k in range(NCORES):
        m = dict(
            xt=XTb,
            pw1=pw_1.astype(bf16), pw2=pw_2.astype(bf16),
            ff1_1=ff1_1, ff2_1=p1["w2"].astype(bf16),
            ff1_2=ff1_2, ff2_2=p2["w2"].astype(bf16),
            b1c_1=b1_1.reshape(128, 1), b2c_1=p1["b2"].reshape(128, 1),
            b1c_2=b1_2.reshape(128, 1), b2c_2=p2["b2"].reshape(128, 1),
            att1=np.tile(np.broadcast_to(p1["att"].reshape(1, 128),
                                         (128, 128)).astype(bf16), (1, 2)),
            att2=np.tile(np.broadcast_to(p2["att"].reshape(1, 128),
                                         (128, 128)).astype(bf16), (1, 2)),
            ident=ident.astype(bf16),
            epsc=np.full((128, 1), EPS, np.float32),
            idx1=plan1["idx_up"][k], idx2=plan2["idx_up"][k],
            oh1=plan1["oh_up"][k], oh2=plan2["oh_up"][k],
        )
        in_maps.append(m)

    key = "nc"
    if key not in _cache:
        _cache[key] = _build_nc(plan1, plan2)
    nc = _cache[key]

    trace = bool(int(os.environ.get("KERNEL_TRACE", "0")))
    res = run_bass_kernel_spmd(nc, in_maps, list(range(NCORES)), trace=trace)
    global last_result
    last_result = res
    outs = np.concatenate([res.results[i]["out"] for i in range(NCORES)],
                          axis=0)
    return outs[pos3].astype(np.float32)


if __name__ == "__main__":
    import reference as ref
    inp = {k: np.asarray(v) for k, v in ref.setup_inputs().items()}
    got = kernel(**inp)
    exp = np.asarray(ref.reference(**inp))
    rel = np.linalg.norm(got - exp) / np.linalg.norm(exp)
    print("rel err:", rel)


# revision 45
# speedup vs baseline: 1.2888x; 1.2888x over previous
"""AllSet hypergraph NN (nn_AllSet_81020263071820) — Trainium2 Bass kernel, v2.

Self-contained: hardcodes shapes for N=100000 nodes, M=800000 incidences,
EH=50000 hyperedges, D=128, H=4 heads. Runs SPMD on 8 NeuronCores.

v2 strategy (vs the AllGather-table baseline):
- Replicate X (bf16) to every core; each core builds the FULL per-node
  message table locally (PE/DVE/ACT are idle anyway) — kills the 4-quarter
  table AllGather (~390us of serialized collective).
- Phase B (V2E) windows produce X1^T tiles; the two window-chunks are
  AllGathered as packed [128, rows] bf16 (1/4 the bytes of the padded
  table) and every core projects the full edge table locally (B2).
- Destination windows per core, dma_gather of 512B table rows into
  slot-buckets, one-hot fp8 matmul scatter-add into PSUM (as baseline),
  but gather calls are 2048-4096 idx (4x fewer GPSIMD SWDGE calls) and
  windows are processed in PAIRS to halve DVE/ACT instruction overheads.
"""
import sys

for _p in ("/opt/trn_rl_repo", "/root/.axon_site", "/root/.axon_site/_ro/pypackages"):
    if _p not in sys.path:
        sys.path.insert(0, _p)

import heapq
import numpy as np
import ml_dtypes

bf16 = ml_dtypes.bfloat16

N = 100000
M1 = 800001
EH1 = 50001
D = 128
H = 4
C = 32
NEG = 0.2
EPS = 1e-5
NCORES = 8
NPAD = 100352        # 784 node tiles; E2V destination rows
EPAD = 50176         # 392 edge windows; V2E destination rows
NW1 = 49             # V2E windows per core
NW2 = 98             # E2V windows per core
SH1 = EPAD // NCORES         # 6272 edge rows per core
SH2 = NPAD // NCORES         # 12544 node rows per core
QSIZE = NPAD // 4            # 25088 rows per table quarter (global)
# per-(core, quarter) block sizes: 128-aligned, alternating so every
# quarter sums to 25088 and every core's blocks sum to 12544
QC_KJ = np.array([[3200 if (k + j) % 2 == 0 else 3072 for j in range(4)]
                  for k in range(NCORES)])
QOFF_KJ = np.concatenate([np.zeros((NCORES, 1), np.int64),
                          np.cumsum(QC_KJ, axis=1)], axis=1)
QROW_KJ = np.concatenate([np.zeros((1, 4), np.int64),
                          np.cumsum(QC_KJ, axis=0)], axis=0)
BOUND = 3136                 # phase-C bucket boundary (local edge row)
CH_ROWS = [3200, 3072]       # x1 chunk cols per core (windows 0..24 / 25..48)
GW1 = 4
GW2 = 8
LIM = 512


# ---------------------------------------------------------------------------
# Host planning (graph-structure only; ported from the validated baseline)
# ---------------------------------------------------------------------------

def _plan_permutations(vtx, edg):
    """Returns pos3 (node->E2V row), pos2 (edge->V2E row),
    (kq, bq, iq) node table coords."""
    n_win2 = NPAD // 128
    deg_v0 = np.bincount(vtx, minlength=N)
    order_v0 = np.argsort(-deg_v0, kind="stable")
    heap3 = [(0, w) for w in range(n_win2)]
    heapq.heapify(heap3)
    w3count = np.zeros(n_win2, np.int64)
    w3tot = np.zeros(n_win2, np.int64)
    pos3 = np.zeros(N, np.int64)
    for v in order_v0:
        while True:
            t, w = heapq.heappop(heap3)
            if w3count[w] < 128:
                break
        pos3[v] = w * 128 + w3count[w]
        w3count[w] += 1
        w3tot[w] += deg_v0[v]
        heapq.heappush(heap3, (int(w3tot[w]), w))

    deg_e = np.bincount(edg, minlength=EH1)
    n_win1 = EPAD // 128
    n_core1 = NW1
    bound = BOUND
    order_e = np.argsort(-deg_e, kind="stable")
    order_ie = np.argsort(edg, kind="stable")
    e_sorted_v = vtx[order_ie]
    e_starts = np.searchsorted(edg[order_ie], np.arange(EH1 + 1))
    cell2 = np.zeros((n_win2, 2), np.int64)
    halfcap = [NCORES * bound, EPAD - NCORES * bound]
    halfcnt = [0, 0]
    half_of_e = np.zeros(EH1, np.int64)
    euw = [None] * EH1
    for e in order_e:
        vws = pos3[e_sorted_v[e_starts[e]:e_starts[e + 1]]] // 128
        uw, cnts = np.unique(vws, return_counts=True)
        euw[e] = (uw, cnts)
        if len(uw):
            o0 = np.maximum(cell2[uw, 0] + cnts - LIM, 0).sum()
            o1 = np.maximum(cell2[uw, 1] + cnts - LIM, 0).sum()
            m0 = (cell2[uw, 0] + cnts).max()
            m1 = (cell2[uw, 1] + cnts).max()
        else:
            o0 = o1 = m0 = m1 = 0
        if halfcnt[0] >= halfcap[0]:
            h = 1
        elif halfcnt[1] >= halfcap[1]:
            h = 0
        else:
            h = 0 if (o0, m0) <= (o1, m1) else 1
        half_of_e[e] = h
        halfcnt[h] += 1
        if len(uw):
            cell2[uw, h] += cnts
    w3_of_inc = pos3[vtx] // 128
    for _ in range(30):
        over = np.argwhere(cell2 > LIM)
        moved = 0
        for w3o, ho in over:
            if cell2[w3o, ho] <= LIM:
                continue
            cand = np.unique(edg[w3_of_inc == w3o])
            contr = []
            for e in cand:
                if half_of_e[e] != ho:
                    continue
                uw, cnts = euw[e]
                c = cnts[uw == w3o]
                contr.append((int(c[0]) if len(c) else 0, int(e)))
            contr.sort(reverse=True)
            for c, e in contr:
                if cell2[w3o, ho] <= LIM:
                    break
                hn = 1 - ho
                if halfcnt[hn] >= halfcap[hn]:
                    continue
                uw, cnts = euw[e]
                if (cell2[uw, hn] + cnts > LIM).any():
                    continue
                cell2[uw, ho] -= cnts
                cell2[uw, hn] += cnts
                halfcnt[ho] -= 1
                halfcnt[hn] += 1
                half_of_e[e] = hn
                moved += 1
        if len(over) == 0 or moved == 0:
            break
    # per-half LPT into windows (flex windows straddle the half boundary)
    heaps = {0: [], 1: []}
    bcap = {}
    for w in range(n_win1):
        lw = w % n_core1
        r0 = lw * 128
        c0 = min(max(bound - r0, 0), 128)
        if c0 > 0:
            bcap[(w, 0)] = c0
            heaps[0].append((0, w))
        if c0 < 128:
            bcap[(w, 1)] = 128 - c0
            heaps[1].append((0, w))
    for h in heaps:
        heapq.heapify(heaps[h])
    wtot = np.zeros(n_win1, np.int64)
    bctr = {k: 0 for k in bcap}
    pos2 = np.zeros(EH1, np.int64)
    for e in order_e:
        h = int(half_of_e[e])
        while True:
            t, w = heapq.heappop(heaps[h])
            if bctr[(w, h)] < bcap[(w, h)]:
                break
        base = 0 if h == 0 else bcap.get((w, 0), 0)
        pos2[e] = w * 128 + base + bctr[(w, h)]
        bctr[(w, h)] += 1
        wtot[w] += deg_e[e]
        heapq.heappush(heaps[h], (int(wtot[w]), w))
    # repair V2E window totals > 4*LIM (cannot split into 4 buckets)
    e_by_win = {}
    for e in range(EH1):
        e_by_win.setdefault(int(pos2[e]) // 128, []).append(e)
    for _ in range(20):
        over_ws = np.where(wtot > 4 * LIM)[0]
        if len(over_ws) == 0:
            break
        swapped = 0
        under_ws = np.argsort(wtot)[:64]
        for wo in over_ws:
            need = int(wtot[wo] - 4 * LIM)
            eo = sorted(e_by_win[int(wo)], key=lambda e: -deg_e[e])
            done = False
            for e1 in eo:
                h1_ = int(half_of_e[e1])
                for wu in under_ws:
                    if wtot[wu] > 4 * LIM - need:
                        continue
                    for e0 in e_by_win[int(wu)]:
                        if int(half_of_e[e0]) != h1_:
                            continue
                        d = int(deg_e[e1] - deg_e[e0])
                        if d >= need and wtot[wu] + d <= 4 * LIM:
                            pos2[e1], pos2[e0] = pos2[e0], pos2[e1]
                            e_by_win[int(wo)].remove(e1)
                            e_by_win[int(wu)].remove(e0)
                            e_by_win[int(wo)].append(e0)
                            e_by_win[int(wu)].append(e1)
                            wtot[wo] -= d
                            wtot[wu] += d
                            swapped += 1
                            done = True
                            break
                    if done:
                        break
                if done:
                    break
        if swapped == 0:
            break
    # greedy node -> quarter (cells (V2E window, quarter) <= LIM)
    dst1 = pos2[edg]
    w1_of_inc = dst1 // 128
    order_inc = np.argsort(vtx, kind="stable")
    v_sorted = vtx[order_inc]
    w_sorted = w1_of_inc[order_inc]
    starts = np.searchsorted(v_sorted, np.arange(N + 1))
    deg_v = starts[1:] - starts[:-1]
    cell = np.zeros((n_win1, 4), np.int32)
    cap_cell = np.zeros((NCORES, 4), np.int32)
    bucket_of_v = np.zeros(N, np.int64)
    order_v = np.argsort(-deg_v, kind="stable")
    for v in order_v:
        ws = w_sorted[starts[v]:starts[v + 1]]
        uw, cnts = np.unique(ws, return_counts=True)
        if len(uw):
            scores = (cell[uw] + cnts[:, None]).max(axis=0)
        else:
            scores = np.zeros(4, np.int64)
        for b in np.argsort(scores, kind="stable"):
            if (cap_cell[:, b] < QC_KJ[:, b]).any():
                break
        bucket_of_v[v] = b
        if len(uw):
            cell[uw, b] += cnts.astype(np.int32)
        k = int(np.argmin(np.where(cap_cell[:, b] < QC_KJ[:, b],
                                   cap_cell[:, b], 10 ** 9)))
        cap_cell[k, b] += 1
    for _ in range(30):
        overc = np.argwhere(cell > LIM)
        movedc = 0
        for w1o, bo in overc:
            if cell[w1o, bo] <= LIM:
                continue
            inc_sel = np.where(w1_of_inc == w1o)[0]
            cand = np.unique(vtx[inc_sel])
            contr = []
            for v in cand:
                if bucket_of_v[v] != bo:
                    continue
                ws = w_sorted[starts[v]:starts[v + 1]]
                contr.append((int((ws == w1o).sum()), int(v)))
            contr.sort(reverse=True)
            for c, v in contr:
                if cell[w1o, bo] <= LIM:
                    break
                ws = w_sorted[starts[v]:starts[v + 1]]
                uw, cnts = np.unique(ws, return_counts=True)
                for bn in range(4):
                    if bn == bo or not (cap_cell[:, bn] < QC_KJ[:, bn]).any():
                        continue
                    if (cell[uw, bn] + cnts > LIM).any():
                        continue
                    cell[uw, bo] -= cnts.astype(np.int32)
                    cell[uw, bn] += cnts.astype(np.int32)
                    ko = int(np.argmax(cap_cell[:, bo]))
                    cap_cell[ko, bo] -= 1
                    kn = int(np.argmin(np.where(
                        cap_cell[:, bn] < QC_KJ[:, bn],
                        cap_cell[:, bn], 10 ** 9)))
                    cap_cell[kn, bn] += 1
                    bucket_of_v[v] = bn
                    movedc += 1
                    break
        if len(overc) == 0 or movedc == 0:
            break
    kq = np.zeros(N, np.int64)
    iq = np.zeros(N, np.int64)
    cap_cell2 = np.zeros((NCORES, 4), np.int64)
    for v in order_v:
        b = bucket_of_v[v]
        k = int(np.argmin(np.where(cap_cell2[:, b] < QC_KJ[:, b],
                                   cap_cell2[:, b], 10 ** 9)))
        kq[v] = k
        iq[v] = cap_cell2[k, b]
        cap_cell2[k, b] += 1
    return pos3, pos2, kq, bucket_of_v, iq, int(cell.max()), int(cell2.max())


def _make_plan(dst, bkt, pos, n_dst_pad, qsizes, gw):
    dst = np.asarray(dst, np.int64)
    bkt = np.asarray(bkt, np.int64)
    pos = np.asarray(pos, np.int64)
    nb = len(qsizes)
    assert all(q < 32768 for q in qsizes)
    per_core = n_dst_pad // NCORES
    n_win = per_core // 128
    core_of = dst // per_core
    win_of = (dst % per_core) // 128
    counts = np.zeros((NCORES, n_win, nb), np.int64)
    np.add.at(counts, (core_of, win_of, bkt), 1)
    cap = int(np.ceil(max(counts.max(), 1) / 128) * 128)
    cpw = cap // 128
    groups = []
    w = 0
    while w < n_win:
        groups.append(min(gw, n_win - w))
        w += gw
    gpre = np.concatenate([[0], np.cumsum(groups)])
    total_slots = n_win * nb * cap
    total_chunks = total_slots // 128
    g_of_win = np.zeros(n_win, np.int64)
    wig_of_win = np.zeros(n_win, np.int64)
    for g, gs in enumerate(groups):
        for wi in range(gs):
            g_of_win[gpre[g] + wi] = g
            wig_of_win[gpre[g] + wi] = wi
    key = (core_of * n_win + win_of) * nb + bkt
    order = np.argsort(key, kind="stable")
    key_s = key[order]
    cell_sizes = np.bincount(key_s, minlength=NCORES * n_win * nb)
    cell_starts = np.concatenate([[0], np.cumsum(cell_sizes)])
    p = np.arange(len(key_s)) - cell_starts[key_s]
    co = core_of[order]
    wo = win_of[order]
    bo = bkt[order]
    gg = g_of_win[wo]
    wig = wig_of_win[wo]
    gs_arr = np.asarray(groups)[gg]
    slot = (gpre[gg] * nb + bo * gs_arr) * cap + wig * cap + p
    idx16 = np.zeros((NCORES, total_slots), np.int16)
    ids = np.full((NCORES, total_chunks, 128), -1.0, np.float32)
    idx16[co, slot] = pos[order].astype(np.int16)
    ids[co, slot // 128, slot % 128] = (dst[order] % 128).astype(np.float32)
    wrapped = idx16.reshape(NCORES, total_slots // 16, 16).transpose(0, 2, 1)
    idx_up = np.tile(wrapped, (1, 8, 1)).astype(np.int16)
    oh = (ids[:, :, :, None] ==
          np.arange(128, dtype=np.float32)[None, None, None, :])
    oh_up = np.ascontiguousarray(
        oh.transpose(0, 2, 1, 3).reshape(NCORES, 128, total_chunks * 128)
    ).astype(ml_dtypes.float8_e4m3)
    return dict(cap=cap, cpw=cpw, n_win=n_win, groups=[int(x) for x in groups],
                gpre=[int(x) for x in gpre], nb=nb,
                idx_up=idx_up, oh_up=oh_up, qsizes=list(qsizes),
                total_slots=total_slots, total_chunks=total_chunks)


def _proj_weights(Kw, Kb, Vw, Vb, att):
    att_f = np.asarray(att, np.float32).reshape(H, C)
    Kw_a = np.zeros((D, H), np.float32)
    Kb_a = np.zeros((H,), np.float32)
    for h in range(H):
        Kw_a[:, h] = np.asarray(Kw, np.float32)[:, h * C:(h + 1) * C] @ att_f[h]
        Kb_a[h] = np.asarray(Kb, np.float32)[h * C:(h + 1) * C] @ att_f[h]
    pw = np.concatenate([np.asarray(Vw, np.float32), Kw_a], axis=1)
    pb = np.concatenate([np.asarray(Vb, np.float32), Kb_a])
    return pw, pb


# ---------------------------------------------------------------------------
# Device graph
# ---------------------------------------------------------------------------

def _build_nc(plan1, plan2):
    import os
    import concourse.bass as bass
    import concourse.bacc as bacc
    import concourse.mybir as mybir
    import concourse.tile as tile

    dt = mybir.dt
    Alu = mybir.AluOpType
    Act = mybir.ActivationFunctionType

    from concourse.hw_specs import get_activation_tables

    nc = bacc.Bacc("TRN2", target_bir_lowering=False, debug=False,
                   num_devices=NCORES, num_swdge_queues=4)
    _tabs = get_activation_tables(nc.m.arch)
    for _k, _v in _tabs.items():
        if _k != "natural_log_exp_and_others":
            _v.clear()

    def ein(name, shape, dty):
        return nc.dram_tensor(name, shape, dty, kind="ExternalInput")

    xt = ein("xt", [128, NPAD], dt.bfloat16)
    pw1 = ein("pw1", [128, 132], dt.bfloat16)
    pw2 = ein("pw2", [128, 132], dt.bfloat16)
    ff1_1 = ein("ff1_1", [128, 128], dt.bfloat16)
    ff2_1 = ein("ff2_1", [128, 128], dt.bfloat16)
    ff1_2 = ein("ff1_2", [128, 128], dt.bfloat16)
    ff2_2 = ein("ff2_2", [128, 128], dt.bfloat16)
    b1c_1 = ein("b1c_1", [128, 1], dt.float32)
    b2c_1 = ein("b2c_1", [128, 1], dt.float32)
    b1c_2 = ein("b1c_2", [128, 1], dt.float32)
    b2c_2 = ein("b2c_2", [128, 1], dt.float32)
    att1 = ein("att1", [128, 256], dt.bfloat16)
    att2 = ein("att2", [128, 256], dt.bfloat16)
    ident = ein("ident", [128, 128], dt.bfloat16)
    epsc = ein("epsc", [128, 1], dt.float32)
    idx1 = ein("idx1", [128, plan1["total_slots"] // 16], dt.int16)
    idx2 = ein("idx2", [128, plan2["total_slots"] // 16], dt.int16)
    oh1 = ein("oh1", [128, plan1["total_chunks"] * 128], dt.float8e4)
    oh2 = ein("oh2", [128, plan2["total_chunks"] * 128], dt.float8e4)
    out = nc.dram_tensor("out", [SH2, 128], dt.float32,
                         kind="ExternalOutput")

    tbl1q = [nc.dram_tensor(f"tbl1q{j}", [QSIZE, 256], dt.bfloat16)
             for j in range(4)]
    x1t_sh = [nc.dram_tensor(f"x1t_sh{c}", [128, CH_ROWS[c]], dt.bfloat16)
              for c in range(2)]
    x1t_f = [nc.dram_tensor(f"x1t_f{c}", [NCORES * 128, CH_ROWS[c]],
                            dt.bfloat16)
             for c in range(2)]
    tbl2b = [nc.dram_tensor(f"tbl2b{c}", [NCORES * BOUND, 256],
                            dt.bfloat16) for c in range(2)]

    cap1, cpw1 = plan1["cap"], plan1["cpw"]
    cap2, cpw2 = plan2["cap"], plan2["cpw"]
    groups1, gpre1 = plan1["groups"], plan1["gpre"]
    groups2, gpre2 = plan2["groups"], plan2["gpre"]

    with tile.TileContext(nc) as tc:
        with tc.tile_pool(name="const", bufs=1) as cp:
            def load_const(name, src_ap, shape, dty):
                t = cp.tile(shape, dty, tag=name)
                nc.sync.dma_start(t[:], src_ap)
                return t

            pw1_t = load_const("pw1", pw1[:], [128, 132], dt.bfloat16)
            pw2_t = load_const("pw2", pw2[:], [128, 132], dt.bfloat16)
            ff11_t = load_const("ff11", ff1_1[:], [128, 128], dt.bfloat16)
            ff21_t = load_const("ff21", ff2_1[:], [128, 128], dt.bfloat16)
            ff12_t = load_const("ff12", ff1_2[:], [128, 128], dt.bfloat16)
            ff22_t = load_const("ff22", ff2_2[:], [128, 128], dt.bfloat16)
            b11_t = load_const("b11", b1c_1[:], [128, 1], dt.float32)
            b21_t = load_const("b21", b2c_1[:], [128, 1], dt.float32)
            b12_t = load_const("b12", b1c_2[:], [128, 1], dt.float32)
            b22_t = load_const("b22", b2c_2[:], [128, 1], dt.float32)
            att1_t = load_const("att1", att1[:], [128, 256], dt.bfloat16)
            att2_t = load_const("att2", att2[:], [128, 256], dt.bfloat16)
            ident_t = load_const("ident", ident[:], [128, 128], dt.bfloat16)
            eps_t = load_const("epsc", epsc[:], [128, 1], dt.float32)
            idx1_t = cp.tile([128, plan1["total_slots"] // 16], dt.int16,
                             tag="idx1")
            nc.sync.dma_start(idx1_t[:], idx1[:])
            idx2_t = cp.tile([128, plan2["total_slots"] // 16], dt.int16,
                             tag="idx2")
            nc.sync.dma_start(idx2_t[:], idx2[:])

            # ---------------- shared helpers --------------------------------
            def build_rows(pool, psp, xsrc, dst_write, pw_t, n, rows=128):
                """Project n (<=3) row-tiles (last may have rows<128):
                psum=[x@Vw | alpha], w=exp(leaky(alpha)), write [xV*w | w].
                xsrc: SBUF AP holding the (n-1)*128+rows source columns."""
                ps = psp.tile([128, 396], dt.float32, tag="bps")
                for j in range(n):
                    r = 128 if j < n - 1 else rows
                    nc.tensor.matmul(ps[0:r, j * 132:(j + 1) * 132],
                                     xsrc[:, j * 128:j * 128 + r],
                                     pw_t[:], start=True, stop=True)
                tb = pool.tile([128, 768], dt.bfloat16, tag="btb")
                tb3 = tb.rearrange("p (t c) -> p t c", c=256)
                ps3 = ps.rearrange("p (t c) -> p t c", c=132)
                w4a = pool.tile([128, 12], dt.float32, tag="bw4a")
                w4av = w4a.rearrange("p (t c) -> p t c", c=4)[:, 0:n, :]
                w4 = pool.tile([128, 12], dt.float32, tag="bw4")
                w4v = w4.rearrange("p (t c) -> p t c", c=4)[:, 0:n, :]
                a_ap = ps3[:, 0:n, 128:132]
                nc.vector.tensor_scalar(w4av, a_ap, NEG, None, Alu.mult)
                nc.vector.tensor_tensor(w4v, w4av, a_ap, Alu.max)
                nc.scalar.activation(tb3[:, 0:n, 128:132], w4v, Act.Exp)
                nc.vector.tensor_tensor(
                    tb3[:, 0:n, 0:128].rearrange("p t (h c2) -> p t h c2",
                                                 c2=C),
                    ps3[:, 0:n, 0:128].rearrange("p t (h c2) -> p t h c2",
                                                 c2=C),
                    tb3[:, 0:n, 128:132].unsqueeze(-1).broadcast_to(
                        [128, n, H, C]),
                    Alu.mult)
                dst_write(tb)

            def pma_pair(pss, n, att_t, ff1_t, ff2_t, b1_t, b2_t, pool, psp,
                         final):
                """PMA epilogue for n (1 or 2) windows, each with its own
                [128,132] psum tile.  Returns [128, n*128] tile."""
                z2 = pool.tile([128, n * 128], dt.bfloat16, tag="z2")
                z23 = z2.rearrange("p (t c) -> p t c", c=128)
                den = pool.tile([128, 8], dt.float32, tag="den")
                rec = pool.tile([128, 8], dt.float32, tag="rec")
                zt = pool.tile([128, n * 128], dt.bfloat16, tag="zt")
                for t in range(n):
                    nc.vector.tensor_scalar(den[:, t * 4:t * 4 + 4],
                                            pss[t][:, 128:132],
                                            1e-16, None, Alu.add)
                    nc.vector.reciprocal(rec[:, t * 4:t * 4 + 4],
                                         den[:, t * 4:t * 4 + 4])
                    nc.vector.tensor_tensor(
                        zt[:, t * 128:(t + 1) * 128].rearrange(
                            "p (h c2) -> p h c2", c2=C),
                        pss[t][:, 0:128].rearrange(
                            "p (h c2) -> p h c2", c2=C),
                        rec[:, t * 4:t * 4 + 4].unsqueeze(-1).broadcast_to(
                            [128, H, C]),
                        Alu.mult)
                nc.vector.tensor_tensor(z2[:, 0:n * 128], zt[:, 0:n * 128],
                                        att_t[:, 0:n * 128], Alu.add)
                st = pool.tile([128, n * 6], dt.float32, tag="st")
                stg = st.rearrange("p (t c) -> p t c", c=6)
                for t in range(n):
                    nc.vector.bn_stats(stg[:, t, :], z23[:, t, :])
                mv = pool.tile([128, n * 2], dt.float32, tag="mv")
                for t in range(n):
                    nc.vector.bn_aggr(mv[:, t * 2:t * 2 + 2], stg[:, t, :])
                lv = pool.tile([128, 2], dt.float32, tag="lv")
                rstd = pool.tile([128, 2], dt.float32, tag="rstd")
                nmr = pool.tile([128, 2], dt.float32, tag="nmr")
                u = pool.tile([128, n * 128], dt.bfloat16, tag="u")
                u3 = u.rearrange("p (t c) -> p t c", c=128)
                for t in range(n):
                    nc.scalar.activation(lv[:, t:t + 1],
                                         mv[:, t * 2 + 1:t * 2 + 2], Act.Ln,
                                         bias=eps_t[:])
                    nc.scalar.activation(rstd[:, t:t + 1], lv[:, t:t + 1],
                                         Act.Exp, scale=-0.5)
                    nc.vector.tensor_scalar(nmr[:, t:t + 1],
                                            mv[:, t * 2:t * 2 + 1],
                                            rstd[:, t:t + 1], -1.0,
                                            Alu.mult, Alu.mult)
                    nc.scalar.activation(u3[:, t, :], z23[:, t, :],
                                         Act.Identity, bias=nmr[:, t:t + 1],
                                         scale=rstd[:, t:t + 1])
                ptx = psp.tile([128, n * 128], dt.bfloat16, tag="trps")
                for t in range(n):
                    nc.tensor.transpose(ptx[:, t * 128:(t + 1) * 128],
                                        u3[:, t, :], ident_t[:])
                uT = pool.tile([128, n * 128], dt.bfloat16, tag="uT")
                nc.vector.tensor_copy(uT[:], ptx[:])
                pf1 = psp.tile([128, n * 128], dt.float32, tag="mmps")
                nc.tensor.matmul(pf1[:], ff1_t[:], uT[:], start=True,
                                 stop=True)
                f1 = pool.tile([128, n * 128], dt.bfloat16, tag="f1")
                nc.scalar.activation(f1[:], pf1[:], Act.Relu, bias=b1_t[:])
                pf2 = psp.tile([128, n * 128], dt.float32, tag="mmps")
                nc.tensor.matmul(pf2[:], ff2_t[:], f1[:], start=True,
                                 stop=True)
                f2T = pool.tile([128, n * 128], dt.bfloat16, tag="f2T")
                nc.scalar.activation(f2T[:], pf2[:], Act.Relu, bias=b2_t[:])
                pt2 = psp.tile([128, n * 128], dt.bfloat16, tag="trps")
                for t in range(n):
                    nc.tensor.transpose(pt2[:, t * 128:(t + 1) * 128],
                                        f2T[:, t * 128:(t + 1) * 128],
                                        ident_t[:])
                r = pool.tile([128, n * 128], dt.bfloat16, tag="r")
                nc.vector.tensor_tensor(r[:], pt2[:], u[:], Alu.add)
                r3 = r.rearrange("p (t c) -> p t c", c=128)
                st2 = pool.tile([128, n * 6], dt.float32, tag="st")
                st2g = st2.rearrange("p (t c) -> p t c", c=6)
                for t in range(n):
                    nc.vector.bn_stats(st2g[:, t, :], r3[:, t, :])
                mv2 = pool.tile([128, n * 2], dt.float32, tag="mv")
                lv2 = pool.tile([128, 2], dt.float32, tag="lv")
                rstd2 = pool.tile([128, 2], dt.float32, tag="rstd")
                nmr2 = pool.tile([128, 2], dt.float32, tag="nmr")
                for t in range(n):
                    nc.vector.bn_aggr(mv2[:, t * 2:t * 2 + 2], st2g[:, t, :])
                    nc.scalar.activation(lv2[:, t:t + 1],
                                         mv2[:, t * 2 + 1:t * 2 + 2],
                                         Act.Ln, bias=eps_t[:])
                    nc.scalar.activation(rstd2[:, t:t + 1], lv2[:, t:t + 1],
                                         Act.Exp, scale=-0.5)
                    nc.vector.tensor_scalar(nmr2[:, t:t + 1],
                                            mv2[:, t * 2:t * 2 + 1],
                                            rstd2[:, t:t + 1], -1.0,
                                            Alu.mult, Alu.mult)
                if final:
                    o = pool.tile([128, n * 128], dt.float32, tag="fin")
                    o3 = o.rearrange("p (t c) -> p t c", c=128)
                    for t in range(n):
                        nc.scalar.activation(o3[:, t, :], r3[:, t, :],
                                             Act.Identity,
                                             bias=nmr2[:, t:t + 1],
                                             scale=rstd2[:, t:t + 1])
                    return o
                x1 = pool.tile([128, n * 128], dt.bfloat16, tag="x1")
                x13 = x1.rearrange("p (t c) -> p t c", c=128)
                for t in range(n):
                    nc.scalar.activation(x13[:, t, :], r3[:, t, :],
                                         Act.Relu, bias=nmr2[:, t:t + 1],
                                         scale=rstd2[:, t:t + 1])
                return x1

            # ---------------- phase A: full local table1 --------------------
            with tc.tile_pool(name="pa", bufs=6) as pa, \
                 tc.tile_pool(name="pa_ps", bufs=3, space="PSUM") as paps:
                for k in range(NCORES):
                    for j in range(4):
                        ntile = int(QC_KJ[k, j]) // 128
                        tq0 = 0
                        while tq0 < ntile:
                            nt12 = min(12, ntile - tq0)
                            col0 = k * SH2 + int(QOFF_KJ[k, j]) + tq0 * 128
                            xbig = pa.tile([128, 1536], dt.bfloat16,
                                           tag="bxt")
                            nc.sync.dma_start(xbig[:, 0:nt12 * 128],
                                              xt[:, col0:col0 + nt12 * 128])
                            tq = tq0
                            while tq < tq0 + nt12:
                                n = min(3, tq0 + nt12 - tq)

                                def writet(tb, j=j, k=k, tq=tq, n=n):
                                    r0 = int(QROW_KJ[k, j]) + tq * 128
                                    dst = tbl1q[j][r0:r0 + n * 128, :]
                                    eng = (nc.scalar if (tq // 3) % 2
                                           else nc.sync)
                                    eng.dma_start(
                                        dst.rearrange("(t p) c -> p t c",
                                                      p=128),
                                        tb.rearrange("p (t c) -> p t c",
                                                     c=256)[:, 0:n, :])

                                off = (tq - tq0) * 128
                                build_rows(pa, paps,
                                           xbig[:, off:off + n * 128],
                                           writet, pw1_t, n)
                                tq += n
                            tq0 += nt12

            # ---------------- phase B: V2E ----------------------------------
            with tc.tile_pool(name="pbg", bufs=2) as pbg, \
                 tc.tile_pool(name="pbo", bufs=3) as pbo, \
                 tc.tile_pool(name="pb", bufs=4) as pb, \
                 tc.tile_pool(name="pb_ps", bufs=2, space="PSUM") as pbps, \
                 tc.tile_pool(name="pb_ag", bufs=2, space="PSUM") as pbag:
                qcount = 0
                for g, gs in enumerate(groups1):
                    nch_g = gs * 4 * cpw1
                    ch0 = gpre1[g] * 4 * cpw1
                    poh = pbo.tile([128, GW1 * 4 * cpw1 * 128], dt.float8e4,
                                   tag="poh")
                    nc.sync.dma_start(poh[:, 0:nch_g * 128],
                                      oh1[:, ch0 * 128:(ch0 + nch_g) * 128])
                    gbs = []
                    for b in range(4):
                        s0 = (gpre1[g] * 4 + b * gs) * cap1
                        nidx = gs * cap1
                        gb = pbg.tile([128, GW1 * cap1 * 2], dt.bfloat16,
                                      tag=f"gb{b}")
                        done = 0
                        while done < nidx:
                            nn_ = min(1024, nidx - done)
                            nc.gpsimd.dma_gather(
                                gb[:, done * 2:(done + nn_) * 2].rearrange(
                                    "p (k e) -> p k e", e=256),
                                tbl1q[b][:, :],
                                idx1_t[:, (s0 + done) // 16:
                                       (s0 + done + nn_) // 16],
                                nn_, nn_, 256, queue_num=qcount % 4)
                            qcount += 1
                            done += nn_
                        gbs.append(gb)
                    wig = 0
                    while wig < gs:
                        n = min(2, gs - wig)
                        ps_a = pbag.tile([128, 512], dt.float32,
                                         tag="agg0")
                        ps_b = pbag.tile([128, 512], dt.float32,
                                         tag="agg1")
                        pss = [ps_a, ps_b][0:n]
                        for b in range(4):
                            for t in range(n):
                                for cc in range(cpw1):
                                    lch = (b * gs + wig + t) * cpw1 + cc
                                    blk = (wig + t) * cpw1 + cc
                                    nc.tensor.matmul(
                                        pss[t][:, 0:132],
                                        poh[:, lch * 128:(lch + 1) * 128],
                                        gbs[b][:, blk * 256:blk * 256 + 132],
                                        start=(b == 0 and cc == 0),
                                        stop=(b == 3 and cc == cpw1 - 1))
                        x1 = pma_pair(pss, n, att1_t, ff11_t, ff21_t, b11_t,
                                      b21_t, pb, pbps, final=False)
                        ptx = pbps.tile([128, n * 128], dt.bfloat16,
                                        tag="trps")
                        x13 = x1.rearrange("p (t c) -> p t c", c=128)
                        for t in range(n):
                            nc.tensor.transpose(ptx[:, t * 128:(t + 1) * 128],
                                                x13[:, t, :], ident_t[:])
                        x1T = pb.tile([128, n * 128], dt.bfloat16, tag="x1T")
                        nc.vector.tensor_copy(x1T[:], ptx[:])
                        lw0 = gpre1[g] + wig
                        if lw0 < 25:
                            # windows 24,25 pair straddles the chunk bound
                            n0 = min(n, 25 - lw0)
                            nc.sync.dma_start(
                                x1t_sh[0][:, lw0 * 128:(lw0 + n0) * 128],
                                x1T[:, 0:n0 * 128])
                            if n0 < n:
                                nc.sync.dma_start(
                                    x1t_sh[1][:, 0:(n - n0) * 128],
                                    x1T[:, n0 * 128:n * 128])
                        else:
                            lcol = (lw0 - 25) * 128
                            nc.sync.dma_start(
                                x1t_sh[1][:, lcol:lcol + n * 128], x1T[:])
                        if lw0 + n == 25 or (lw0 < 25 <= lw0 + n):
                            nc.gpsimd.collective_compute(
                                "AllGather", Alu.bypass,
                                replica_groups=[list(range(NCORES))],
                                ins=[x1t_sh[0].ap().opt()],
                                outs=[x1t_f[0].ap().opt()])
                        wig += n
                nc.gpsimd.collective_compute(
                    "AllGather", Alu.bypass,
                    replica_groups=[list(range(NCORES))],
                    ins=[x1t_sh[1].ap().opt()],
                    outs=[x1t_f[1].ap().opt()])

            # ---------------- phase B2: full local table2 -------------------
            # tbl2b[h] rows k*3136 + i; source x1 local row l = h*3136 + i:
            # l < 3200 -> x1t_f[0][k,:,l], else x1t_f[1][k,:,l-3200]
            with tc.tile_pool(name="pb2", bufs=6) as pb2, \
                 tc.tile_pool(name="pb2_ps", bufs=3, space="PSUM") as pb2ps:
                for h in range(2):
                    for k in range(NCORES):
                        segs = []          # (row0, nrows-per-tile list, ...)
                        i = 0
                        while i < BOUND:
                            l = h * BOUND + i
                            if l < 3200:
                                lim_seg = min(3200 - l, BOUND - i)
                                src = 0
                                scol = l
                            else:
                                lim_seg = BOUND - i
                                src = 1
                                scol = l - 3200
                            take = min(384, lim_seg)
                            ntile = (take + 127) // 128
                            lastr = take - (ntile - 1) * 128

                            x2t = pb2.tile([128, 384], dt.bfloat16,
                                           tag="b2x")
                            nc.sync.dma_start(
                                x2t[:, 0:take],
                                x1t_f[src][k * 128:(k + 1) * 128,
                                           scol:scol + take])

                            def writet2(tb, h=h, k=k, i=i, ntile=ntile,
                                        lastr=lastr):
                                r0 = k * BOUND + i
                                nf = ntile if lastr == 128 else ntile - 1
                                eng = nc.scalar if (i // 384) % 2 else nc.sync
                                if nf:
                                    dst = tbl2b[h][r0:r0 + nf * 128, :]
                                    eng.dma_start(
                                        dst.rearrange("(t p) c -> p t c",
                                                      p=128),
                                        tb.rearrange("p (t c) -> p t c",
                                                     c=256)[:, 0:nf, :])
                                if nf < ntile:
                                    eng.dma_start(
                                        tbl2b[h][r0 + nf * 128:
                                                 r0 + nf * 128 + lastr, :],
                                        tb[0:lastr,
                                           nf * 256:(nf + 1) * 256])

                            build_rows(pb2, pb2ps, x2t[:, 0:take],
                                       writet2, pw2_t, ntile, rows=lastr)
                            i += take

            # ---------------- phase C: E2V ----------------------------------
            with tc.tile_pool(name="pcg", bufs=2) as pcg, \
                 tc.tile_pool(name="pco", bufs=3) as pco, \
                 tc.tile_pool(name="pc", bufs=4) as pc, \
                 tc.tile_pool(name="pc_ps", bufs=2, space="PSUM") as pcps, \
                 tc.tile_pool(name="pc_ag", bufs=2, space="PSUM") as pcag:
                qcount = 0
                for g, gs in enumerate(groups2):
                    nch_g = gs * 2 * cpw2
                    ch0 = gpre2[g] * 2 * cpw2
                    poh = pco.tile([128, GW2 * 2 * cpw2 * 128], dt.float8e4,
                                   tag="poh2")
                    nc.sync.dma_start(poh[:, 0:nch_g * 128],
                                      oh2[:, ch0 * 128:(ch0 + nch_g) * 128])
                    gbs = []
                    for b in range(2):
                        s0 = (gpre2[g] * 2 + b * gs) * cap2
                        nidx = gs * cap2
                        gb = pcg.tile([128, GW2 * cap2 * 2], dt.bfloat16,
                                      tag=f"gc{b}")
                        done = 0
                        while done < nidx:
                            nn_ = min(1024, nidx - done)
                            nc.gpsimd.dma_gather(
                                gb[:, done * 2:(done + nn_) * 2].rearrange(
                                    "p (k e) -> p k e", e=256),
                                tbl2b[b][:, :],
                                idx2_t[:, (s0 + done) // 16:
                                       (s0 + done + nn_) // 16],
                                nn_, nn_, 256, queue_num=qcount % 4)
                            qcount += 1
                            done += nn_
                        gbs.append(gb)
                    wig = 0
                    while wig < gs:
                        n = min(2, gs - wig)
                        ps_a = pcag.tile([128, 512], dt.float32,
                                         tag="agg20")
                        ps_b = pcag.tile([128, 512], dt.float32,
                                         tag="agg21")
                        pss = [ps_a, ps_b][0:n]
                        for b in range(2):
                            for t in range(n):
                                for cc in range(cpw2):
                                    lch = (b * gs + wig + t) * cpw2 + cc
                                    blk = (wig + t) * cpw2 + cc
                                    nc.tensor.matmul(
                                        pss[t][:, 0:132],
                                        poh[:, lch * 128:(lch + 1) * 128],
                                        gbs[b][:, blk * 256:blk * 256 + 132],
                                        start=(b == 0 and cc == 0),
                                        stop=(b == 1 and cc == cpw2 - 1))
                        o = pma_pair(pss, n, att2_t, ff12_t, ff22_t, b12_t,
                                     b22_t, pc, pcps, final=True)
                        w0 = gpre2[g] + wig
                        for t in range(n):
                            nc.sync.dma_start(
                                out[(w0 + t) * 128:(w0 + t + 1) * 128, :],
                                o[:, t * 128:(t + 1) * 128])
                        wig += n

    nc.finalize()
    return nc


# ---------------------------------------------------------------------------
# Entry point
# ---------------------------------------------------------------------------

_cache = {}
last_result = None


def kernel(**inputs):
    import os
    from concourse.bass_utils import run_bass_kernel_spmd

    X = np.asarray(inputs["X"], np.float32)
    vertex = np.asarray(inputs["vertex"], np.int64)
    edges = np.asarray(inputs["edges"], np.int64)
    vtx = np.concatenate([vertex, [N - 1]])
    edg = np.concatenate([edges, [EH1 - 1]])

    def P(prefix):
        return {k: np.asarray(inputs[f"{prefix}_{k}"], np.float32)
                for k in ("Kw", "Kb", "Vw", "Vb", "att", "w1", "b1", "w2",
                          "b2", "ln0s", "ln0b", "ln1s", "ln1b")}

    p1, p2 = P("v2e"), P("e2v")

    import hashlib
    hsh = hashlib.md5(vtx.tobytes() + edg.tobytes()).hexdigest()[:12]
    pcache = f"/tmp/allset_plan_{hsh}.npz"
    try:
        zc = np.load(pcache)
        pos3, pos2, kq, bq, iq = (zc["pos3"], zc["pos2"], zc["kq"],
                                  zc["bq"], zc["iq"])
        cm1 = cm2 = -1
    except Exception:
        pos3, pos2, kq, bq, iq, cm1, cm2 = _plan_permutations(vtx, edg)
        try:
            np.savez(pcache, pos3=pos3, pos2=pos2, kq=kq, bq=bq, iq=iq)
        except Exception:
            pass
    print(f"planner cellmax: phaseB={cm1} phaseC={cm2}", file=sys.stderr)

    pos1 = QROW_KJ[kq, bq] + iq                  # row within quarter tensor
    plan1 = _make_plan(pos2[edg], bq[vtx], pos1[vtx],
                       EPAD, [QSIZE] * 4, GW1)
    k_of = pos2[edg] // SH1
    l_of = pos2[edg] % SH1
    bkt2 = (l_of >= BOUND).astype(np.int64)
    pos_t2 = k_of * BOUND + (l_of % BOUND)
    plan2 = _make_plan(pos3[vtx], bkt2, pos_t2, NPAD,
                       [NCORES * BOUND] * 2, GW2)
    print(f"plan caps: V2E={plan1['cap']} E2V={plan2['cap']} "
          f"slots={plan1['total_slots']}/{plan2['total_slots']}",
          file=sys.stderr)

    pw_1, pb_1 = _proj_weights(p1["Kw"], p1["Kb"], p1["Vw"], p1["Vb"],
                               p1["att"])
    pw_2, pb_2 = _proj_weights(p2["Kw"], p2["Kb"], p2["Vw"], p2["Vb"],
                               p2["att"])
    assert np.all(pb_1 == 0) and np.all(pb_2 == 0), \
        "nonzero projection biases not supported by this kernel build"
    for p in (p1, p2):
        assert np.all(p["ln0s"] == 1) and np.all(p["ln0b"] == 0)
        assert np.all(p["ln1s"] == 1) and np.all(p["ln1b"] == 0)
        assert np.all(p["b1"] == 0) and np.all(p["b2"] == 0)

    ff1_1 = (np.diag(p1["ln0s"]) @ p1["w1"]).astype(bf16)
    ff1_2 = (np.diag(p2["ln0s"]) @ p2["w1"]).astype(bf16)
    b1_1 = (p1["ln0b"] @ p1["w1"] + p1["b1"]).astype(np.float32)
    b1_2 = (p2["ln0b"] @ p2["w1"] + p2["b1"]).astype(np.float32)

    # XT column for node v: core kq, quarter bq at QOFF_KJ, index iq
    xcol = kq * SH2 + QOFF_KJ[kq, bq] + iq
    XT = np.zeros((128, NPAD), np.float32)
    XT[:, xcol] = X.T
    XTb = XT.astype(bf16)
    ident = np.eye(128, dtype=np.float32)

    in_maps = []
    for k in range(NCORES):
        m = dict(
            xt=XTb,
            pw1=pw_1.astype(bf16), pw2=pw_2.astype(bf16),
            ff1_1=ff1_1, ff2_1=p1["w2"].astype(bf16),
            ff1_2=ff1_2, ff2_2=p2["w2"].astype(bf16),
            b1c_1=b1_1.reshape(128, 1), b2c_1=p1["b2"].reshape(128, 1),
            b1c_2=b1_2.reshape(128, 1), b2c_2=p2["b2"].reshape(128, 1),
            att1=np.tile(np.broadcast_to(p1["att"].reshape(1, 128),
                                         (128, 128)).astype(bf16), (1, 2)),
            att2=np.tile(np.broadcast_to(p2["att"].reshape(1, 128),
                                         (128, 128)).astype(bf16), (1, 2)),
            ident=ident.astype(bf16),
            epsc=np.full((128, 1), EPS, np.float32),
            idx1=plan1["idx_up"][k], idx2=plan2["idx_up"][k],
            oh1=plan1["oh_up"][k], oh2=plan2["oh_up"][k],
        )
        in_maps.append(m)

    key = "nc"
    if key not in _cache:
        _cache[key] = _build_nc(plan1, plan2)
    nc = _cache[key]

    trace = bool(int(os.environ.get("KERNEL_TRACE", "0")))
    res = run_bass_kernel_spmd(nc, in_maps, list(range(NCORES)), trace=trace)
    global last_result
    last_result = res
    outs = np.concatenate([res.results[i]["out"] for i in range(NCORES)],
                          axis=0)
    return outs[pos3].astype(np.float32)


if __name__ == "__main__":
    import reference as ref
    inp = {k: np.asarray(v) for k, v in ref.setup_inputs().items()}
    got = kernel(**inp)
    exp = np.asarray(ref.reference(**inp))
    rel = np.linalg.norm(got - exp) / np.linalg.norm(exp)
    print("rel err:", rel)
